# revision 3
# baseline (speedup 1.0000x reference)
"""Mean-IoU kernel for Trainium2, SPMD over 8 NeuronCores.

Strategy (data-parallel over batch N=16, 2 images per core):
  - Host pre-transposes inputs to (N, H, W, C) so the on-chip layout is
    pixels-on-partitions with classes innermost: x tile (128, F, 19) f32,
    fully contiguous for DMA (19456B runs), reduce and subtract.
  - Per tile: m = reduce_max over the contiguous class axis (DVE), then
    d = x - m (broadcast) -> bf16, split across DVE and GpSimd to
    balance engine load, then s = Sign(d) in {-1, 0} on the otherwise
    idle Scalar engine (Sign(0)==0 verified on HW).
  - TensorE bf16 matmuls Sp^T @ St accumulate block "sign products"
    (JB=6 pixel-columns -> 114x114 PSUM) per image.
  - Host: sum jb-diagonal 19x19 blocks -> S_ss; recover the confusion
    matrix exactly via the row-sum identity (each sign row sums to -18):
    conf = S_ss - rowsum/18 - colsum/18 + Npix; then IoU + means.
"""
import os
import sys

for _p in ('/opt/trn_rl_repo', '/root/.axon_site/_ro/trn_rl_repo'):
    if os.path.isdir(_p) and _p not in sys.path:
        sys.path.insert(0, _p)

import numpy as np

# problem constants (hardcoded per contest rules)
N_FULL = 16
C = 19
H = 512
W = 512
HW = H * W
EPS = 1e-06

N_CORES = 8
N_LOC = N_FULL // N_CORES      # 2 images per core
P = 128                        # SBUF partitions = pixel groups
Q = HW // P                    # 2048 pixels per partition
F = 256                        # pixels per partition per tile
N_TILES = Q // F               # 8 tiles per image
JB = 6                         # pixel-columns per confusion matmul
NCOLS = JB * C                 # 114
# of the 32 x-tiles per core, how many run the subtract on DVE
# (the rest go to GpSimd); ACT does all the signs either way.
A_DVE = int(os.environ.get("KERNEL_A_DVE", "12"))

_CACHE = {}


def _build_nc():
    from concourse import bacc, tile, mybir

    nc = bacc.Bacc("TRN2", target_bir_lowering=False, debug=False,
                   num_devices=N_CORES)
    # host-transposed layout: (n, h*w, c)
    preds = nc.dram_tensor("preds", (N_LOC, HW, C), mybir.dt.float32,
                           kind="ExternalInput")
    targs = nc.dram_tensor("targets", (N_LOC, HW, C), mybir.dt.float32,
                           kind="ExternalInput")
    conf_out = nc.dram_tensor("conf", (N_LOC, NCOLS, NCOLS), mybir.dt.float32,
                              kind="ExternalOutput")

    pv = preds.ap().rearrange("n (p j) c -> n p j c", p=P)
    tv = targs.ap().rearrange("n (p j) c -> n p j c", p=P)

    # spread the A_DVE DVE-subtract tiles evenly over the 32 x-tiles
    n_tt = 2 * N_TILES * N_LOC
    dve_route = []
    acc = 0
    for i in range(n_tt):
        nacc = (i + 1) * A_DVE // n_tt
        dve_route.append(nacc > acc)
        acc = nacc

    with tile.TileContext(nc) as tc:
        with (
            tc.tile_pool(name="sbuf", bufs=2) as pool,
            tc.tile_pool(name="psum", bufs=2, space="PSUM") as psum_pool,
        ):
            tt_idx = 0
            for n in range(N_LOC):
                conf = psum_pool.tile([NCOLS, NCOLS], mybir.dt.float32)
                for t in range(N_TILES):
                    stiles = {}
                    for name, dview in (("p", pv), ("t", tv)):
                        x = pool.tile([P, F, C], mybir.dt.float32,
                                      tag=f"x{name}")
                        nc.sync.dma_start(x[:], dview[n, :,
                                                      t * F:(t + 1) * F, :])
                        m = pool.tile([P, F], mybir.dt.float32, tag=f"m{name}")
                        nc.vector.reduce_max(m[:], x[:],
                                             axis=mybir.AxisListType.X)
                        mb = m[:, :, None].broadcast_to((P, F, C))
                        d = pool.tile([P, F, C], mybir.dt.bfloat16,
                                      tag=f"d{name}")
                        if dve_route[tt_idx]:
                            nc.vector.tensor_tensor(
                                d[:], x[:], mb, op=mybir.AluOpType.subtract)
                        else:
                            nc.gpsimd.tensor_tensor(
                                d[:], x[:], mb, op=mybir.AluOpType.subtract)
                        s = pool.tile([P, F, C], mybir.dt.bfloat16,
                                      tag=f"s{name}")
                        nc.scalar.sign(
                            s[:].rearrange("p j c -> p (j c)"),
                            d[:].rearrange("p j c -> p (j c)"))
                        stiles[name] = s
                        tt_idx += 1
                    spf = stiles["p"][:].rearrange("p j c -> p (j c)")
                    stf = stiles["t"][:].rearrange("p j c -> p (j c)")
                    nmm = (F + JB - 1) // JB            # 43 (42 full + 1 of 4)
                    for b in range(nmm):
                        cols = min(JB, F - b * JB) * C
                        first = (t == 0 and b == 0)
                        last = (t == N_TILES - 1 and b == nmm - 1)
                        nc.tensor.matmul(conf[0:cols, 0:cols],
                                         spf[:, b * NCOLS:b * NCOLS + cols],
                                         stf[:, b * NCOLS:b * NCOLS + cols],
                                         start=first, stop=last)
                sb = pool.tile([NCOLS, NCOLS], mybir.dt.float32, tag="confsb")
                nc.scalar.copy(sb[:], conf[:])
                nc.sync.dma_start(conf_out.ap()[n], sb[:])

    nc.compile()
    return nc


def _get_nc():
    if "nc" not in _CACHE:
        _CACHE["nc"] = _build_nc()
    return _CACHE["nc"]


def run_on_hw(preds, targets, trace=False):
    """Run the SPMD kernel; returns (conf (16,NCOLS,NCOLS) np.f32, results)."""
    from concourse.bass_utils import run_bass_kernel_spmd

    nc = _get_nc()
    # (N, C, H, W) -> (N, H*W, C) contiguous, so every device access is
    # contiguous (DMA runs, class-axis reduce, subtract/sign writes)
    preds = np.ascontiguousarray(
        np.asarray(preds, dtype=np.float32).reshape(N_FULL, C, HW)
        .transpose(0, 2, 1))
    targets = np.ascontiguousarray(
        np.asarray(targets, dtype=np.float32).reshape(N_FULL, C, HW)
        .transpose(0, 2, 1))
    in_maps = [
        {"preds": preds[i * N_LOC:(i + 1) * N_LOC],
         "targets": targets[i * N_LOC:(i + 1) * N_LOC]}
        for i in range(N_CORES)
    ]
    res = run_bass_kernel_spmd(nc, in_maps, core_ids=list(range(N_CORES)),
                               trace=trace)
    conf = np.concatenate([res.results[i]["conf"] for i in range(N_CORES)],
                          axis=0)
    return conf, res


def postprocess(conf, class_weights):
    """conf: (16, NCOLS, NCOLS) block sign-products -> scalar mean IoU.

    S_ss[c1,c2] = sum_pix sp[pix,c1]*st[pix,c2] with s in {-1,0}.
    Each sign row sums to exactly -18 (one zero at the argmax), so with
    zp = 1+sp: conf = S_ss + rs1 + r1t + Npix where
    rs1[c1] = sum_pix sp[c1] = -S_ss.sum(axis=c2)/18 (exact), etc.
    """
    conf = conf.astype(np.float64)
    S = np.zeros((N_FULL, C, C))
    for k in range(JB):
        S += conf[:, k * C:(k + 1) * C, k * C:(k + 1) * C]
    npix = float(HW)
    s_s1 = -S.sum(axis=2) / 18.0                      # (N, C)
    s_1t = -S.sum(axis=1) / 18.0                      # (N, C)
    M = S + s_s1[:, :, None] + s_1t[:, None, :] + npix
    inter = np.diagonal(M, axis1=1, axis2=2)          # (N, C)
    pred_cnt = M.sum(axis=2)                          # (N, C)
    targ_cnt = M.sum(axis=1)                          # (N, C)
    union = pred_cnt + targ_cnt - inter
    iou = (inter + EPS) / (union + EPS)
    weighted = iou * np.asarray(class_weights, dtype=np.float64)[None, :]
    return np.float32(weighted.mean())


def kernel(preds, targets, class_weights):
    conf, _ = run_on_hw(preds, targets, trace=False)
    return postprocess(conf, class_weights)


# revision 6
# speedup vs baseline: 1.5825x; 1.5825x over previous
"""Mean-IoU kernel for Trainium2, SPMD over 8 NeuronCores.

Strategy (data-parallel over batch N=16, 2 images per core), v4:
  - NO host transpose: inputs stay class-planar (N, C, H*W), the native
    HBM layout. SWDGE DMA casts f32 -> bf16 on the way in, so on-chip
    tiles are x (128, 19, FT) bf16 with PIXELS innermost.
  - Per-pixel max over classes = chain of 18 DVE tensor_tensor(max) ops
    on contiguous (128, FT) bf16 slices. Both operands are step-1 bf16
    -> DVE 2x_1P mode (2 elem/cycle), 2x faster than a 1x tensor_reduce
    over a class-innermost layout.
  - One-hot zb = is_equal(x, m) with m broadcast along the MIDDLE class
    axis: inner step stays 1 on both operands -> 2x mode again.
    bf16 rounding can produce multi-hot rows (~0.7% of pixels); the
    resulting mean-IoU error is ~5e-5 (validated numerically), far
    under the 2e-2 tolerance.
  - TensorE bf16 matmuls Zp^T @ Zt accumulate a block confusion matrix:
    JB=4 pixel-columns x 19 classes -> 76x76 PSUM per image, columns
    ordered class-major (c*JB + j).
  - Host: sum j-diagonal of (19,4,19,4) blocks -> confusion M;
    pred = M.sum(1), targ = M.sum(0), inter = diag(M); IoU + means.
"""
import os
import sys

for _p in ('/opt/trn_rl_repo', '/root/.axon_site/_ro/trn_rl_repo'):
    if os.path.isdir(_p) and _p not in sys.path:
        sys.path.insert(0, _p)

import numpy as np

# problem constants (hardcoded per contest rules)
N_FULL = 16
C = 19
H = 512
W = 512
HW = H * W
EPS = 1e-06

N_CORES = 8
N_LOC = N_FULL // N_CORES      # 2 images per core
P = 128                        # SBUF partitions = pixel groups
Q = HW // P                    # 2048 pixels per partition
FT = 512                       # pixels per partition per chunk-tile
N_TILES = Q // FT              # 4 chunks per image
JB = 4                         # pixel-columns per confusion matmul
NCOLS = JB * C                 # 76

_CACHE = {}


def _build_nc():
    from concourse import bacc, tile, mybir

    nc = bacc.Bacc("TRN2", target_bir_lowering=False, debug=False,
                   num_devices=N_CORES)
    # native layout: (n, c, h*w)
    preds = nc.dram_tensor("preds", (N_LOC, C, HW), mybir.dt.float32,
                           kind="ExternalInput")
    targs = nc.dram_tensor("targets", (N_LOC, C, HW), mybir.dt.float32,
                           kind="ExternalInput")
    conf_out = nc.dram_tensor("conf", (N_LOC, NCOLS, NCOLS), mybir.dt.float32,
                              kind="ExternalOutput")

    # (n, c, p, q): partition p holds pixels [p*Q, (p+1)*Q) of each plane
    pvv = preds.ap().rearrange("n c (p q) -> n p c q", p=P)
    tvv = targs.ap().rearrange("n c (p q) -> n p c q", p=P)

    with tile.TileContext(nc) as tc:
        with (
            tc.tile_pool(name="sbuf", bufs=2) as pool,
            tc.tile_pool(name="psum", bufs=2, space="PSUM") as psum_pool,
        ):
            for n in range(N_LOC):
                conf = psum_pool.tile([NCOLS, NCOLS], mybir.dt.float32)
                for t in range(N_TILES):
                    zbs = {}
                    for name, dview in (("p", pvv), ("t", tvv)):
                        x = pool.tile([P, C, FT], mybir.dt.bfloat16,
                                      tag=f"x{name}")
                        # SWDGE DMA with f32 -> bf16 cast
                        nc.gpsimd.dma_start(
                            x[:], dview[n, :, :, t * FT:(t + 1) * FT])
                        m = pool.tile([P, FT], mybir.dt.bfloat16,
                                      tag=f"m{name}")
                        nc.vector.tensor_copy(m[:], x[:, 0, :])
                        for c in range(1, C):
                            nc.vector.tensor_tensor(
                                m[:], m[:], x[:, c, :],
                                op=mybir.AluOpType.max)
                        # micro-tiled one-hot: (p, nb, c, j) so each JB-pixel
                        # block is a contiguous 76-column slab for the PE,
                        # while the DVE writes through a permuted view that
                        # still streams in input order with inner step 1.
                        zb = pool.tile([P, FT // JB, C, JB],
                                       mybir.dt.bfloat16, tag=f"zb{name}")
                        mb = m[:, None, :].broadcast_to((P, C, FT))
                        nc.vector.tensor_tensor(
                            zb[:].rearrange("p nb c j -> p c nb j"),
                            x[:], mb, op=mybir.AluOpType.is_equal)
                        zbs[name] = zb
                    nmm = FT // JB                    # 128 uniform blocks
                    for b in range(nmm):
                        first = (t == 0 and b == 0)
                        last = (t == N_TILES - 1 and b == nmm - 1)
                        nc.tensor.matmul(
                            conf[:],
                            zbs["p"][:, b].rearrange("p c j -> p (c j)"),
                            zbs["t"][:, b].rearrange("p c j -> p (c j)"),
                            start=first, stop=last)
                sb = pool.tile([NCOLS, NCOLS], mybir.dt.float32, tag="confsb")
                nc.scalar.copy(sb[:], conf[:])
                nc.sync.dma_start(conf_out.ap()[n], sb[:])

    nc.compile()
    return nc


def _get_nc():
    if "nc" not in _CACHE:
        _CACHE["nc"] = _build_nc()
    return _CACHE["nc"]


def run_on_hw(preds, targets, trace=False):
    """Run the SPMD kernel; returns (conf (16,NCOLS,NCOLS) np.f32, results)."""
    from concourse.bass_utils import run_bass_kernel_spmd

    nc = _get_nc()
    preds = np.ascontiguousarray(
        np.asarray(preds, dtype=np.float32).reshape(N_FULL, C, HW))
    targets = np.ascontiguousarray(
        np.asarray(targets, dtype=np.float32).reshape(N_FULL, C, HW))
    in_maps = [
        {"preds": preds[i * N_LOC:(i + 1) * N_LOC],
         "targets": targets[i * N_LOC:(i + 1) * N_LOC]}
        for i in range(N_CORES)
    ]
    res = run_bass_kernel_spmd(nc, in_maps, core_ids=list(range(N_CORES)),
                               trace=trace)
    conf = np.concatenate([res.results[i]["conf"] for i in range(N_CORES)],
                          axis=0)
    return conf, res


def postprocess(conf, class_weights):
    """conf: (16, NCOLS, NCOLS) block confusion -> scalar mean IoU.

    Column index = c*JB + j (class-major within a JB-pixel block);
    the per-class confusion sums the j-diagonal.
    """
    conf = conf.astype(np.float64).reshape(N_FULL, C, JB, C, JB)
    M = np.einsum('ncjdj->ncd', conf)
    inter = np.diagonal(M, axis1=1, axis2=2)          # (N, C)
    pred_cnt = M.sum(axis=2)                          # (N, C)
    targ_cnt = M.sum(axis=1)                          # (N, C)
    union = pred_cnt + targ_cnt - inter
    iou = (inter + EPS) / (union + EPS)
    weighted = iou * np.asarray(class_weights, dtype=np.float64)[None, :]
    return np.float32(weighted.mean())


def kernel(preds, targets, class_weights):
    conf, _ = run_on_hw(preds, targets, trace=False)
    return postprocess(conf, class_weights)
